# revision 1
# baseline (speedup 1.0000x reference)
"""Multi-head causal attention on 8 Trainium2 NeuronCores.

Problem: x[4,2048,1024] @ {W_q,W_k,W_v}, 16 heads x d_k=64, causal softmax,
context @ W_o. Sharding: 8 cores = 4 batches x 2 head-groups (tensor
parallel over heads, data parallel over batch). Each core computes, for its
batch b and its 8 heads: projections, causal attention, and a partial
output  context_g @ W_o[g-rows]  [2048,1024]. Host sums the two partials
per batch (the W_o row-split reduction) and stacks batches.

Layout strategy (everything contraction-major; single x transpose):
  xT[D,S]   via PE-transpose of x
  QT[dd,S] = Wq_g.T x.T   (lhsT=Wq chunks, rhs=xT)      f32r
  KT[dd,S], V[S,dd] likewise; V augmented with a ones column per head so
      the context matmul's row 64 yields the softmax denominator l free
  ST[k,q] -> PSUM pairs [128k, 2, 512q];  E = exp(ST/8) one ACT op per
      pair; causal mask via gpsimd affine_select on diagonal halves;
      matmul/exp free dims trimmed to the causal range (floor 256)
  ctxT[65,q] accumulated over k-blocks (lhsT=V_aug, rhs=E halves)
  1/l via partition-spread DVE reciprocal + gpsimd partition_broadcast,
      DVE multiply; ctxT to per-chunk DRAM scratch
  out[q,1024] accumulated over 4 ctx chunks (lhsT=ctxT chunk, rhs=Wo_g)

Schedule: attention is exp(ACT)-throughput-paced, so projection work for
sequence-quarter q+1 is interleaved between attention groups of query-tile
q to keep the PE array busy (idle PE triggers the HAM clock-gate to half
rate, which doubles matmul time for the whole phase). The projection pools
are freed after the qt=2 phase so W_o and the ctx lhsT can be prefetched
into SBUF during the qt=3 phase, removing the DRAM-roundtrip stall before
the output projection.
"""
from contextlib import ExitStack

import numpy as np

import concourse.bacc as bacc
import concourse.mybir as mybir
import concourse.tile as tile
from concourse.bass_utils import run_bass_kernel_spmd
from concourse.masks import make_identity

P = 128
S = 2048
D = 1024
GW = 512          # per-core head-group width (8 heads x 64)
DK = 64
HG = 8
NDC = D // P
NQT = S // 512
NSB = S // P
NCH = GW // P

F32 = mybir.dt.float32
F32R = mybir.dt.float32r
RDT = F32R
SCALE = 0.125
N_CORES = 8


def build():
    nc = bacc.Bacc("TRN2", target_bir_lowering=False, debug=False)
    xb = nc.dram_tensor("xb", [S, D], F32, kind="ExternalInput")
    wq = nc.dram_tensor("wq", [D, GW], F32, kind="ExternalInput")
    wk = nc.dram_tensor("wk", [D, GW], F32, kind="ExternalInput")
    wv = nc.dram_tensor("wv", [D, GW], F32, kind="ExternalInput")
    wo = nc.dram_tensor("wo", [GW, D], F32, kind="ExternalInput")
    outp = nc.dram_tensor("outp", [S, D], F32, kind="ExternalOutput")

    def r(ap):
        return ap.bitcast(RDT) if RDT is F32R else ap

    with tile.TileContext(nc) as tc, \
         tc.tile_pool(name="const", bufs=1) as cpool, \
         tc.tile_pool(name="dram", bufs=1, space="DRAM") as dpool, \
         tc.tile_pool(name="stores", bufs=1) as stores, \
         tc.tile_pool(name="qtp", bufs=2) as qtp, \
         tc.tile_pool(name="e", bufs=6) as epool, \
         tc.tile_pool(name="lwork", bufs=1) as lwork, \
         tc.tile_pool(name="cstage", bufs=2) as cstage, \
         tc.tile_pool(name="ps_sc", bufs=2, space="PSUM") as ps_sc, \
         tc.tile_pool(name="ps_cx", bufs=2, space="PSUM") as ps_cx, \
         tc.tile_pool(name="ps_pj", bufs=2, space="PSUM") as ps_pj:

        proj_stack = ExitStack()
        wpool = proj_stack.enter_context(tc.tile_pool(name="wqkv", bufs=1))
        xin = proj_stack.enter_context(tc.tile_pool(name="xin", bufs=2))
        xtp = proj_stack.enter_context(tc.tile_pool(name="xt", bufs=1))

        ident = cpool.tile([P, P], F32, tag="ident")
        make_identity(nc, ident[:])

        kT = stores.tile([P, NCH, S], RDT, tag="kT")
        v_aug = stores.tile([P, NSB, HG, DK + 1], RDT, tag="v")
        nc.vector.tensor_copy(
            v_aug[:, :, :, DK:DK + 1],
            nc.const_aps.tensor(1.0, (P, NSB, HG, 1), F32))
        ctx_dram = [dpool.tile([P, S], F32, name=f"ctxd{c}", tag=f"ctxd{c}")
                    for c in range(NCH)]
        qT_tiles = {}
        xt_cur = {}
        ctxl = {}

        # ---- projection emission units for one sequence-quarter ----------
        def proj_units(q4):
            units = []

            def load_w():
                # on the ACT hwdge queue: parallel with x loads on sync
                wq_t = wpool.tile([P, NDC, GW], RDT, tag="wq")
                wk_t = wpool.tile([P, NDC, GW], RDT, tag="wk")
                wv_t = wpool.tile([P, NDC, GW], RDT, tag="wv")
                for j in range(NCH):
                    nc.scalar.dma_start(
                        wk_t[:, :, j * P:(j + 1) * P],
                        r(wk[:, j * P:(j + 1) * P].rearrange("(dc p) n -> p dc n", p=P)))
                for j in range(NCH):
                    nc.scalar.dma_start(
                        wq_t[:, :, j * P:(j + 1) * P],
                        r(wq[:, j * P:(j + 1) * P].rearrange("(dc p) n -> p dc n", p=P)))
                nc.scalar.dma_start(wv_t[:], r(wv.rearrange("(dc p) n -> p dc n", p=P)))
                proj_units.w = (wq_t, wk_t, wv_t)

            def start():
                xt_cur[0] = xtp.tile([P, NDC, 512], RDT, tag="xt", name=f"xt{q4}")
                qT_tiles[q4] = qtp.tile([P, NCH, 512], RDT, tag="qT", name=f"qT{q4}")
            units.append(start)

            def transpose_block(sbl):
                xt_q = xt_cur[0]
                sb = q4 * 4 + sbl
                x_blk = xin.tile([P, D], F32, tag="xin")
                nc.sync.dma_start(x_blk[:], xb[sb * P:(sb + 1) * P, :])
                for g in range(2):
                    tp_ps = ps_pj.tile([P, 4, P], F32, tag="pj")
                    for i in range(4):
                        dc = g * 4 + i
                        nc.tensor.transpose(
                            tp_ps[:, i, :],
                            x_blk[:, dc * P:(dc + 1) * P], ident[:])
                    nc.vector.tensor_copy(
                        xt_q[:, g * 4:(g + 1) * 4, sbl * P:(sbl + 1) * P],
                        tp_ps[:].bitcast(F32))
            for sbl in range(4):
                units.append(lambda sbl=sbl: transpose_block(sbl))
                if q4 == 0 and sbl == 0:
                    units.append(load_w)

            def qk_proj(w_i, j):
                w_t = proj_units.w[w_i]
                dst = qT_tiles[q4] if w_i == 0 else kT
                pj = ps_pj.tile([P, 512], F32, tag="pj")
                for dc in range(NDC):
                    nc.tensor.matmul(pj[:], w_t[:, dc, j * P:(j + 1) * P],
                                     xt_cur[0][:, dc, :],
                                     start=(dc == 0), stop=(dc == NDC - 1))
                if w_i == 0:
                    nc.vector.tensor_copy(dst[:, j, :], pj[:].bitcast(F32))
                else:
                    nc.vector.tensor_copy(
                        dst[:, j, q4 * 512:(q4 + 1) * 512], pj[:].bitcast(F32))

            def v_proj(sbl):
                sb = q4 * 4 + sbl
                pj = ps_pj.tile([P, 512], F32, tag="pj")
                for dc in range(NDC):
                    nc.tensor.matmul(pj[:], xt_cur[0][:, dc, sbl * P:(sbl + 1) * P],
                                     proj_units.w[2][:, dc, :],
                                     start=(dc == 0), stop=(dc == NDC - 1))
                nc.vector.tensor_copy(v_aug[:, sb, :, :DK], pj[:].bitcast(F32))

            for j in range(NCH):
                units.append(lambda j=j: qk_proj(1, j))   # K first
            for j in range(NCH):
                units.append(lambda j=j: qk_proj(0, j))   # then Q
            for sbl in range(4):
                units.append(lambda sbl=sbl: v_proj(sbl))
            return units

        # ---- attention group emitters ------------------------------------
        def vstart(kb, qt):
            # first causally-valid q in the tile for k-block kb, capped so
            # trimmed matmul free dims stay >= 256 (f32r fast regime)
            return min(max(0, P * (kb - 4 * qt)), 256)

        def emit_scores(h, qt):
            po = 64 * (h % 2)
            j = h // 2
            q_ap = qT_tiles[qt][po:po + 64, j, :]
            kt_h = kT[po:po + 64, j, :]
            e_pairs = []
            for pr in range(2 * (qt + 1)):
                vs0 = vstart(2 * pr, qt)
                s_ps = ps_sc.tile([P, 2, 512], F32, tag="sc")
                for i in range(2):
                    kb = 2 * pr + i
                    vs = vstart(kb, qt)
                    nc.tensor.matmul(s_ps[:, i, vs:],
                                     kt_h[:, kb * P:(kb + 1) * P],
                                     q_ap[:, vs:], start=True, stop=True)
                e_sb = epool.tile([P, 2, 512], RDT, tag="e")
                nc.scalar.activation(e_sb[:, :, vs0:], s_ps[:, :, vs0:],
                                     mybir.ActivationFunctionType.Exp,
                                     scale=SCALE)
                for i in range(2):
                    kb = 2 * pr + i
                    if kb >= 4 * qt:
                        # zero the below-diagonal part and stale-exp overhang
                        nc.gpsimd.affine_select(
                            out=e_sb[:, i, vs0:], in_=e_sb[:, i, vs0:],
                            compare_op=mybir.AluOpType.is_ge,
                            fill=0.0, base=512 * qt - kb * P + vs0,
                            pattern=[[1, 512 - vs0]], channel_multiplier=-1)
                e_pairs.append(e_sb)
            return e_pairs

        def emit_ctx(h, qt, e_pairs):
            po = 64 * (h % 2)
            j = h // 2
            nk = 4 * (qt + 1)
            ctx_ps = ps_cx.tile([P, 512], F32, tag="cx")
            for kb in range(nk):
                vs = vstart(kb, qt)
                nc.tensor.matmul(ctx_ps[0:DK + 1, vs:],
                                 v_aug[:, kb, h, :],
                                 e_pairs[kb // 2][:, kb % 2, vs:],
                                 start=(kb == 0), stop=(kb == nk - 1),
                                 skip_group_check=True)
            # 1/l: spread 512 l values over 8 partitions so the DVE
            # reciprocal (serial within a partition) is cheap, gather back,
            # broadcast over the 64 ctx partitions.
            lrow = lwork.tile([P, 512], F32, tag="lrow")
            nc.vector.tensor_copy(lrow[64:65, :], ctx_ps[64:65, :])
            lsp = lwork.tile([P, 512], F32, tag="lsp")
            nc.sync.dma_start(lsp[0:8, 0:64], lrow[64:65, :])
            nc.vector.reciprocal(lsp[0:8, 64:128], lsp[0:8, 0:64])
            linv = lwork.tile([P, 512], F32, tag="linv")
            nc.sync.dma_start(linv[0:1, :], lsp[0:8, 64:128])
            lrep = lwork.tile([P, 512], F32, tag="lrep")
            nc.gpsimd.partition_broadcast(lrep[0:DK, :], linv[0:1, :],
                                          channels=DK)
            stg = cstage.tile([P, 512], F32, tag="stg")
            nc.vector.tensor_mul(out=stg[0:DK, :], in0=ctx_ps[0:DK, :],
                                 in1=lrep[0:DK, :])
            nc.sync.dma_start(
                ctx_dram[j][po:po + 64, qt * 512:(qt + 1) * 512],
                stg[0:DK, :])
            if qt == NQT - 1 and "t" in ctxl:
                # mirror the freshly written slice into the resident lhsT
                nc.sync.dma_start(
                    ctxl["t"][po:po + 64, j, qt * 512:(qt + 1) * 512],
                    r(ctx_dram[j][po:po + 64, qt * 512:(qt + 1) * 512]))

        # ---- interleaved emission: quarters 0..2 + attention qt 0..2 -----
        for u in proj_units(0):
            u()
        prev = None
        for qt in range(NQT - 1):
            pu = proj_units(qt + 1)
            pi = 0
            for h in range(HG):
                e_pairs = emit_scores(h, qt)
                if prev is not None:
                    emit_ctx(*prev)
                prev = (h, qt, e_pairs)
                take = ((h + 1) * len(pu)) // HG - (h * len(pu)) // HG
                for _ in range(take):
                    pu[pi]()
                    pi += 1
        proj_stack.close()

        # ---- qt=3 attention + resident out-projection inputs -------------
        with tc.tile_pool(name="wo", bufs=1) as wop, \
             tc.tile_pool(name="clhs", bufs=1) as clhs, \
             tc.tile_pool(name="ostage", bufs=3) as ostage:
            emit_ctx(*prev)        # (h7, qt2): last qt<=2 ctx write
            prev = None
            wo_t = wop.tile([P, NCH, D], RDT, tag="wo")
            nc.sync.dma_start(wo_t[:], r(wo.rearrange("(c p) n -> p c n", p=P)))
            ctx_l = clhs.tile([P, NCH, S], RDT, tag="ctxl")
            ctxl["t"] = ctx_l
            for c in range(NCH):   # bulk-prefetch the qt<=2 regions
                nc.sync.dma_start(ctx_l[:, c, 0:1536],
                                  r(ctx_dram[c][:, 0:1536]))
            def out_group(qb, nh):
                po_ps = ps_pj.tile([P, 512], F32, tag="pj")
                for c in range(NCH):
                    nc.tensor.matmul(
                        po_ps[:], ctx_l[:, c, qb * P:(qb + 1) * P],
                        wo_t[:, c, nh * 512:(nh + 1) * 512],
                        start=(c == 0), stop=(c == NCH - 1))
                ost = ostage.tile([P, 512], F32, tag="ost")
                nc.vector.tensor_copy(ost[:], po_ps[:])
                nc.sync.dma_start(
                    outp[qb * P:(qb + 1) * P, nh * 512:(nh + 1) * 512],
                    ost[:])

            # out-proj groups whose q rows lie in qt<=2: ready now; weave 3
            # per attention group as PE fill for the exp-paced qt=3 phase
            early = [(qb, nh) for qb in range(12) for nh in range(2)]
            qt = NQT - 1
            ei = 0
            for h in range(HG):
                e_pairs = emit_scores(h, qt)
                if prev is not None:
                    emit_ctx(*prev)
                prev = (h, qt, e_pairs)
                take = ((h + 1) * len(early)) // HG - (h * len(early)) // HG
                for _ in range(take):
                    out_group(*early[ei])
                    ei += 1
            emit_ctx(*prev)
            for qb in range(12, NSB):
                for nh in range(2):
                    out_group(qb, nh)
    nc.compile()
    return nc


_NC_CACHE = None


def _get_nc():
    global _NC_CACHE
    if _NC_CACHE is None:
        _NC_CACHE = build()
    return _NC_CACHE


def _run(x, W_q, W_k, W_v, W_o, trace=False, tmpdir=None):
    x = np.ascontiguousarray(x, dtype=np.float32)
    W_q = np.ascontiguousarray(W_q, dtype=np.float32)
    W_k = np.ascontiguousarray(W_k, dtype=np.float32)
    W_v = np.ascontiguousarray(W_v, dtype=np.float32)
    W_o = np.ascontiguousarray(W_o, dtype=np.float32)
    B = x.shape[0]
    in_maps = []
    for c in range(N_CORES):
        b, g = c // 2, c % 2
        in_maps.append({
            "xb": x[b],
            "wq": np.ascontiguousarray(W_q[:, g * GW:(g + 1) * GW]),
            "wk": np.ascontiguousarray(W_k[:, g * GW:(g + 1) * GW]),
            "wv": np.ascontiguousarray(W_v[:, g * GW:(g + 1) * GW]),
            "wo": np.ascontiguousarray(W_o[g * GW:(g + 1) * GW, :]),
        })
    nc = _get_nc()
    res = run_bass_kernel_spmd(nc, in_maps, core_ids=list(range(N_CORES)),
                               trace=trace, tmpdir=tmpdir)
    out = np.empty((B, S, D), np.float32)
    for b in range(B):
        out[b] = res.results[2 * b]["outp"] + res.results[2 * b + 1]["outp"]
    return out, res


def kernel(x, W_q, W_k, W_v, W_o):
    out, _ = _run(x, W_q, W_k, W_v, W_o)
    return out



# revision 4
# speedup vs baseline: 1.3194x; 1.3194x over previous
"""Multi-head causal attention on 8 Trainium2 NeuronCores.

Problem: x[4,2048,1024] @ {W_q,W_k,W_v}, 16 heads x d_k=64, causal softmax,
context @ W_o. Sharding: 8 cores = 4 batches x 2 head-groups (tensor
parallel over heads, data parallel over batch). Each core computes, for its
batch b and its 8 heads: projections, causal attention, and a partial
output  context_g @ W_o[g-rows]  [2048,1024]. Host sums the two partials
per batch (the W_o row-split reduction) and stacks batches.

v2 (bf16): all matmul operands in bf16 (inputs host-cast). Wins vs the
f32r baseline: FWL fast weight loads (LDWEIGHTS was 223us), exact causal
trims (no 256-column floor), 1-cycle/row transposes, halved DMA + SBUF so
the out-projection lhsT (ctx_l) is resident from the start (no DRAM
scratch roundtrip). Score matmuls are packed two-heads-at-a-time: the
even head lives on PE row-groups 0-1 (partitions 0-63) and the odd head
on row-groups 2-3 (partitions 64-127); issued adjacently they execute
concurrently, halving the score phase. Ctx keeps the M=65 ones-row trick
(l rides along free) - col-packing would lose the denominator.

Layout (contraction-major; single x transpose):
  xT[D,S]     PE-transpose of x (bf16, 8 chunks per sb-block in one bank)
  QT/KT[dd,S] = W.T x.T    V[S,dd] + ones column per head
  ST[k,q]     pair PSUM [128k, 2head, 512q]; E = exp(ST/8) one ACT per
              k-block covering both heads; diag mask = fixed 128-wide
              triangle (base=0, j>=k) on gpsimd
  ctxT[65,q]  accumulated over k-blocks per head; staged to SBUF bf16
              immediately (frees PSUM), then scaled by 1/l (partition-
              spread DVE reciprocal + gpsimd broadcast) into resident
              ctx_l
  out[q,1024] = sum_c ctx_l chunks @ W_o, streamed per 128-row group

Schedule: ACT (exp) throughput paces attention, so fill work is woven
between attention groups: projections for quarter qt+1 and the output
projection for quarter qt-1 both interleave into attention quarter qt.
"""
import numpy as np

import concourse.bacc as bacc
import concourse.mybir as mybir
import concourse.tile as tile
from concourse.bass_utils import run_bass_kernel_spmd
from concourse.masks import make_identity

P = 128
S = 2048
D = 1024
GW = 512          # per-core head-group width (8 heads x 64)
DK = 64
HG = 8
NHP = HG // 2     # head pairs (even head rows 0-63, odd head rows 64-127)
NDC = D // P
NQT = S // 512
NSB = S // P
NCH = GW // P

F32 = mybir.dt.float32
BF = mybir.dt.bfloat16
SCALE = 0.125
N_CORES = 8


def vstart(kb, qt):
    # first causally-valid q in the 512-wide query tile for k-block kb
    return min(max(0, P * (kb - 4 * qt)), 384)


def build():
    nc = bacc.Bacc("TRN2", target_bir_lowering=False, debug=False)
    xb = nc.dram_tensor("xb", [S, D], BF, kind="ExternalInput")
    wq = nc.dram_tensor("wq", [D, GW], BF, kind="ExternalInput")
    wk = nc.dram_tensor("wk", [D, GW], BF, kind="ExternalInput")
    wv = nc.dram_tensor("wv", [D, GW], BF, kind="ExternalInput")
    wo = nc.dram_tensor("wo", [GW, D], BF, kind="ExternalInput")
    outp = nc.dram_tensor("outp", [S, D], F32, kind="ExternalOutput")

    with tile.TileContext(nc) as tc, \
         tc.tile_pool(name="const", bufs=1) as cpool, \
         tc.tile_pool(name="stores", bufs=1) as stores, \
         tc.tile_pool(name="wqkv", bufs=1) as wpool, \
         tc.tile_pool(name="xin", bufs=2) as xin, \
         tc.tile_pool(name="xt", bufs=1) as xtp, \
         tc.tile_pool(name="qtp", bufs=2) as qtp, \
         tc.tile_pool(name="e", bufs=24) as epool, \
         tc.tile_pool(name="lwork", bufs=1) as lwork, \
         tc.tile_pool(name="cstage", bufs=3) as cstage, \
         tc.tile_pool(name="ostage", bufs=3) as ostage, \
         tc.tile_pool(name="ps_sc", bufs=2, space="PSUM") as ps_sc, \
         tc.tile_pool(name="ps_cx", bufs=2, space="PSUM") as ps_cx, \
         tc.tile_pool(name="ps_pj", bufs=2, space="PSUM") as ps_pj:

        ident = cpool.tile([P, P], BF, tag="ident")
        make_identity(nc, ident[:])

        kT = stores.tile([P, NCH, S], BF, tag="kT")
        v_aug = stores.tile([P, NSB, HG, DK + 1], BF, tag="v")
        nc.vector.tensor_copy(
            v_aug[:, :, :, DK:DK + 1],
            nc.const_aps.tensor(1.0, (P, NSB, HG, 1), F32))
        ctx_l = stores.tile([P, NCH, S], BF, tag="ctxl")
        wo_t = stores.tile([P, NCH, D], BF, tag="wo")
        qT_tiles = {}
        xt_cur = {}

        # ---- projection emission units for one sequence-quarter ----------
        def proj_units(q4):
            units = []

            def load_w():
                # on the ACT hwdge queue: parallel with x loads on sync
                wq_t = wpool.tile([P, NDC, GW], BF, tag="wq")
                wk_t = wpool.tile([P, NDC, GW], BF, tag="wk")
                wv_t = wpool.tile([P, NDC, GW], BF, tag="wv")
                for j in range(NCH):
                    nc.scalar.dma_start(
                        wk_t[:, :, j * P:(j + 1) * P],
                        wk[:, j * P:(j + 1) * P].rearrange("(dc p) n -> p dc n", p=P))
                for j in range(NCH):
                    nc.scalar.dma_start(
                        wq_t[:, :, j * P:(j + 1) * P],
                        wq[:, j * P:(j + 1) * P].rearrange("(dc p) n -> p dc n", p=P))
                nc.scalar.dma_start(wv_t[:], wv.rearrange("(dc p) n -> p dc n", p=P))
                nc.scalar.dma_start(wo_t[:], wo.rearrange("(c p) n -> p c n", p=P))
                proj_units.w = (wq_t, wk_t, wv_t)

            def start():
                xt_cur[0] = xtp.tile([P, NDC, 512], BF, tag="xt", name=f"xt{q4}")
                qT_tiles[q4] = qtp.tile([P, NCH, 512], BF, tag="qT", name=f"qT{q4}")
            units.append(start)

            def transpose_block(sbl):
                xt_q = xt_cur[0]
                sb = q4 * 4 + sbl
                x_blk = xin.tile([P, D], BF, tag="xin")
                nc.sync.dma_start(x_blk[:], xb[sb * P:(sb + 1) * P, :])
                tp_ps = ps_pj.tile([P, NDC, P], BF, tag="pj")
                for dc in range(NDC):
                    nc.tensor.transpose(
                        tp_ps[:, dc, :], x_blk[:, dc * P:(dc + 1) * P], ident[:])
                nc.vector.tensor_copy(
                    xt_q[:, :, sbl * P:(sbl + 1) * P], tp_ps[:])
            for sbl in range(4):
                units.append(lambda sbl=sbl: transpose_block(sbl))
                if q4 == 0 and sbl == 0:
                    units.append(load_w)

            def qk_proj(w_i, j):
                w_t = proj_units.w[w_i]
                dst = qT_tiles[q4] if w_i == 0 else kT
                pj = ps_pj.tile([P, 512], F32, tag="pj")
                for dc in range(NDC):
                    nc.tensor.matmul(pj[:], w_t[:, dc, j * P:(j + 1) * P],
                                     xt_cur[0][:, dc, :],
                                     start=(dc == 0), stop=(dc == NDC - 1))
                if w_i == 0:
                    nc.vector.tensor_copy(dst[:, j, :], pj[:])
                else:
                    nc.vector.tensor_copy(
                        dst[:, j, q4 * 512:(q4 + 1) * 512], pj[:])

            def v_proj(sbl):
                sb = q4 * 4 + sbl
                pj = ps_pj.tile([P, 512], F32, tag="pj")
                for dc in range(NDC):
                    nc.tensor.matmul(pj[:], xt_cur[0][:, dc, sbl * P:(sbl + 1) * P],
                                     proj_units.w[2][:, dc, :],
                                     start=(dc == 0), stop=(dc == NDC - 1))
                nc.vector.tensor_copy(v_aug[:, sb, :, :DK], pj[:])

            for j in range(NCH):
                units.append(lambda j=j: qk_proj(1, j))   # K first
            for j in range(NCH):
                units.append(lambda j=j: qk_proj(0, j))   # then Q
            for sbl in range(4):
                units.append(lambda sbl=sbl: v_proj(sbl))
            return units

        # ---- attention group emitters (two heads 2hp, 2hp+1 at once) -----
        def emit_scores(hp, qt):
            e_blocks = []
            for kb in range(4 * (qt + 1)):
                vs = vstart(kb, qt)
                s_ps = ps_sc.tile([P, 2, 512], F32, tag="sc")
                for slot in range(2):
                    po = 64 * slot
                    nc.tensor.matmul(s_ps[:, slot, vs:],
                                     kT[po:po + 64, hp, kb * P:(kb + 1) * P],
                                     qT_tiles[qt][po:po + 64, hp, vs:],
                                     start=True, stop=True)
                e_sb = epool.tile([P, 2, 512], BF, tag="e")
                nc.scalar.activation(e_sb[:, :, vs:], s_ps[:, :, vs:],
                                     mybir.ActivationFunctionType.Exp,
                                     scale=SCALE)
                if kb >= 4 * qt:
                    # diagonal block: zero below-diagonal; in the 128-wide
                    # window starting at vs the mask is always keep j >= k
                    nc.gpsimd.affine_select(
                        out=e_sb[:, :, vs:vs + P], in_=e_sb[:, :, vs:vs + P],
                        compare_op=mybir.AluOpType.is_ge,
                        fill=0.0, base=0,
                        pattern=[[0, 2], [1, P]], channel_multiplier=-1)
                e_blocks.append((e_sb, vs))
            return e_blocks

        def emit_ctx(hp, qt, e_blocks):
            nk = 4 * (qt + 1)
            ctx_ps = [ps_cx.tile([P, 512], F32, tag="cx", name=f"cx{slot}")
                      for slot in range(2)]
            for kb in range(nk):
                e_sb, vs = e_blocks[kb]
                for slot in range(2):
                    nc.tensor.matmul(ctx_ps[slot][0:DK + 1, vs:],
                                     v_aug[:, kb, 2 * hp + slot, :],
                                     e_sb[:, slot, vs:],
                                     start=(kb == 0), stop=(kb == nk - 1),
                                     skip_group_check=True)
            for slot in range(2):
                po = 64 * slot
                # stage ctx+l out of PSUM immediately to free the bank
                stg = cstage.tile([P, 512], BF, tag="stg")
                nc.vector.tensor_copy(stg[0:DK + 1, :], ctx_ps[slot][0:DK + 1, :])
                # 1/l: spread the 512 l values over 8 partitions so the DVE
                # reciprocal (serial within a partition) is cheap, gather
                # back, broadcast over the DK ctx partitions.
                lsp = lwork.tile([P, 512], BF, tag="lsp")
                nc.sync.dma_start(lsp[0:8, 0:64], stg[DK:DK + 1, :])
                lre = lwork.tile([P, 512], F32, tag="lre")
                nc.vector.reciprocal(lre[0:8, 0:64], lsp[0:8, 0:64])
                linv = lwork.tile([P, 512], F32, tag="linv")
                nc.sync.dma_start(linv[0:1, :], lre[0:8, 0:64])
                lrep = lwork.tile([P, 512], F32, tag="lrep")
                nc.gpsimd.partition_broadcast(lrep[0:DK, :], linv[0:1, :],
                                              channels=DK)
                nc.vector.tensor_mul(
                    out=ctx_l[po:po + DK, hp, qt * 512:(qt + 1) * 512],
                    in0=stg[0:DK, :], in1=lrep[0:DK, :])

        def out_group(qb, nh):
            po_ps = ps_pj.tile([P, 512], F32, tag="pj")
            for c in range(NCH):
                nc.tensor.matmul(
                    po_ps[:], ctx_l[:, c, qb * P:(qb + 1) * P],
                    wo_t[:, c, nh * 512:(nh + 1) * 512],
                    start=(c == 0), stop=(c == NCH - 1))
            ost = ostage.tile([P, 512], F32, tag="ost")
            nc.vector.tensor_copy(ost[:], po_ps[:])
            nc.sync.dma_start(
                outp[qb * P:(qb + 1) * P, nh * 512:(nh + 1) * 512],
                ost[:])

        # ---- interleaved emission ----------------------------------------
        for u in proj_units(0):
            u()
        prev = None
        for qt in range(NQT):
            pu = proj_units(qt + 1) if qt < NQT - 1 else []
            if qt >= 1:
                # output projection for quarter qt-1 (ctx complete once the
                # last head-pair of qt-1 is emitted, at hp=0 below)
                oq = qt - 1
                pu += [(lambda qb=qb, nh=nh: out_group(qb, nh))
                       for qb in range(4 * oq, 4 * oq + 4) for nh in range(2)]
            pi = 0
            for hp in range(NHP):
                e_blocks = emit_scores(hp, qt)
                if prev is not None:
                    emit_ctx(*prev)
                prev = (hp, qt, e_blocks)
                take = ((hp + 1) * len(pu)) // NHP - (hp * len(pu)) // NHP
                for _ in range(take):
                    pu[pi]()
                    pi += 1
        emit_ctx(*prev)
        for qb in range(S // P - 4, S // P):   # out-proj for the last quarter
            for nh in range(2):
                out_group(qb, nh)
    nc.compile()
    return nc


_NC_CACHE = None


def _get_nc():
    global _NC_CACHE
    if _NC_CACHE is None:
        _NC_CACHE = build()
    return _NC_CACHE


def _run(x, W_q, W_k, W_v, W_o, trace=False, tmpdir=None):
    import ml_dtypes
    bf16 = ml_dtypes.bfloat16
    x = np.asarray(x).astype(bf16)
    W_q = np.asarray(W_q).astype(bf16)
    W_k = np.asarray(W_k).astype(bf16)
    W_v = np.asarray(W_v).astype(bf16)
    W_o = np.asarray(W_o).astype(bf16)
    B = x.shape[0]
    in_maps = []
    for c in range(N_CORES):
        b, g = c // 2, c % 2
        in_maps.append({
            "xb": np.ascontiguousarray(x[b]),
            "wq": np.ascontiguousarray(W_q[:, g * GW:(g + 1) * GW]),
            "wk": np.ascontiguousarray(W_k[:, g * GW:(g + 1) * GW]),
            "wv": np.ascontiguousarray(W_v[:, g * GW:(g + 1) * GW]),
            "wo": np.ascontiguousarray(W_o[g * GW:(g + 1) * GW, :]),
        })
    nc = _get_nc()
    res = run_bass_kernel_spmd(nc, in_maps, core_ids=list(range(N_CORES)),
                               trace=trace, tmpdir=tmpdir)
    out = np.empty((B, S, D), np.float32)
    for b in range(B):
        out[b] = res.results[2 * b]["outp"] + res.results[2 * b + 1]["outp"]
    return out, res


def kernel(x, W_q, W_k, W_v, W_o):
    out, _ = _run(x, W_q, W_k, W_v, W_o)
    return out
